# revision 10
# baseline (speedup 1.0000x reference)
"""Trainium2 Bass kernel for a 4-layer GQA transformer LM (nn_CustomLLM_35278861369705).

Sharding: sequence-parallel across 8 cores — 2 batch groups x 4 sequence chunks
of 256 tokens. Activations kept transposed [feature, token] on device.

v2: all matmul operands bf16 (PSUM accumulation fp32, residual stream fp32),
host pre-lays every weight into its exact SBUF tile layout so each weight
matrix is ONE contiguous large DMA (4-32KB/partition descriptors at line
rate), full-layer weights single-buffered and prefetched a layer ahead, MLP
weights streamed in double-buffered 4-ftile chunks, K/V computed before Q so
the group AllGather overlaps Q-projection, and the softmax denominator is
fused into the attention o-matmul via a ones-column appended to V.
"""
import numpy as np
import ml_dtypes

import concourse.bass as bass
import concourse.mybir as mybir
import concourse.tile as tile
from concourse import bacc
from concourse.bass_utils import run_bass_kernel_spmd

V, H, NH, KVH, I, L, S, B = 32000, 1024, 16, 4, 4096, 4, 1024, 2
HD = 64
THETA = 10000.0
EPS = 1e-5
T = 256            # tokens per core
NCORE = 8
GROUPS = [[0, 1, 2, 3], [4, 5, 6, 7]]
VSH = V // 4       # vocab shard per core (within its 4-core group)
KT = H // 128      # 8
IT = I // 128      # 32
NVC = 16           # vocab chunks per core
VC = VSH // NVC    # 500
FCH = 4            # MLP f-tiles per streamed chunk
NCH = IT // FCH    # 8

F32 = mybir.dt.float32
BF = mybir.dt.bfloat16
AF = mybir.ActivationFunctionType
BF_NP = ml_dtypes.bfloat16

_CACHE = {}


def build_program():
    nc = bacc.Bacc("TRN2", target_bir_lowering=False, debug=False,
                   num_devices=NCORE)

    # ---------------- I/O ----------------
    x0 = nc.dram_tensor("x0", [128, KT, T], F32, kind="ExternalInput").ap()
    cos2 = nc.dram_tensor("cos2", [128, T], F32, kind="ExternalInput").ap()
    sin2 = nc.dram_tensor("sin2", [128, T], F32, kind="ExternalInput").ap()
    ropeR = nc.dram_tensor("ropeR", [128, 128], BF, kind="ExternalInput").ap()
    ones_in = nc.dram_tensor("ones_in", [128, 128], BF, kind="ExternalInput").ap()
    mask_in = nc.dram_tensor("mask", [128, 8, T], BF, kind="ExternalInput").ap()
    emb_in = nc.dram_tensor("embT", [128, NVC, KT, VC], BF, kind="ExternalInput").ap()
    wq_d, wk_d, wv_d, wo_d, wg_d, wu_d, wd_d = [], [], [], [], [], [], []
    for l in range(L):
        wq_d.append(nc.dram_tensor(f"wq{l}", [128, KT, KT, 128], BF, kind="ExternalInput").ap())
        wk_d.append(nc.dram_tensor(f"wk{l}", [128, KT, 2, 128], BF, kind="ExternalInput").ap())
        wv_d.append(nc.dram_tensor(f"wv{l}", [128, KT, 256], BF, kind="ExternalInput").ap())
        wo_d.append(nc.dram_tensor(f"wo{l}", [64, NH, KT, 128], BF, kind="ExternalInput").ap())
        wg_d.append(nc.dram_tensor(f"wg{l}", [128, IT, KT, 128], BF, kind="ExternalInput").ap())
        wu_d.append(nc.dram_tensor(f"wu{l}", [128, IT, KT, 128], BF, kind="ExternalInput").ap())
        wd_d.append(nc.dram_tensor(f"wd{l}", [128, IT, KT, 128], BF, kind="ExternalInput").ap())
    logits = nc.dram_tensor("logits", [S, VSH], BF, kind="ExternalOutput").ap()

    with tile.TileContext(nc) as tc:
        with (
            tc.tile_pool(name="const", bufs=1) as cpool,
            tc.tile_pool(name="xres", bufs=1) as xpool,
            tc.tile_pool(name="hnorm", bufs=2) as hpool,
            tc.tile_pool(name="sqp", bufs=2) as sqpool,
            tc.tile_pool(name="tmps", bufs=3) as tpool,
            tc.tile_pool(name="dram", bufs=2, space="DRAM") as dpool,
        ):
            # ---- persistent constants ----
            cos_sb = cpool.tile([128, T], F32, tag="cos")
            sin_sb = cpool.tile([128, T], F32, tag="sin")
            nc.scalar.dma_start(cos_sb[:], cos2[:])
            nc.scalar.dma_start(sin_sb[:], sin2[:])
            ropeR_sb = cpool.tile([128, 128], BF, tag="ropeR")
            nc.scalar.dma_start(ropeR_sb[:], ropeR[:])
            ones_sb = cpool.tile([128, 128], BF, tag="ones")
            nc.scalar.dma_start(ones_sb[:], ones_in[:])
            mask_sb = cpool.tile([128, 8, T], BF, tag="mask")
            nc.scalar.dma_start(mask_sb[:], mask_in[:])

            # ---- residual stream ----
            xT = xpool.tile([128, KT, T], F32, tag="xT")
            nc.scalar.dma_start(xT[:], x0[:])

            def rmsnorm(src, sq=None):
                """src: [128, KT, T] f32 -> hT [128, KT, T] bf16 (norm weights are
                folded into the following matmul weights on host). If sq (the
                elementwise square of src) was already streamed out slice-wise,
                pass it in to skip the full-tile squaring here."""
                if sq is None:
                    sq = sqpool.tile([128, KT, T], BF, tag="sq")
                    nc.vector.tensor_mul(out=sq[:], in0=src[:], in1=src[:])
                with tc.tile_pool(name="psnorm", bufs=1, space="PSUM") as pp:
                    ps = pp.tile([128, T], F32, tag="ps_norm")
                    for kt in range(KT):
                        nc.tensor.matmul(ps[:], ones_sb[:], sq[:, kt],
                                         start=(kt == 0), stop=(kt == KT - 1))
                    ms = tpool.tile([128, T], F32, tag="ms")
                    nc.scalar.activation(ms[:], ps[:], AF.Copy, bias=EPS, scale=1.0 / H)
                rcp = tpool.tile([128, T], F32, tag="rcp")
                nc.vector.reciprocal(rcp[:], ms[:])
                inv = tpool.tile([128, T], F32, tag="inv")
                nc.scalar.activation(inv[:], rcp[:], AF.Sqrt)
                hT = hpool.tile([128, KT, T], BF, tag="h")
                nc.vector.tensor_mul(out=hT[:], in0=src[:],
                                     in1=inv[:, None, :].to_broadcast((128, KT, T)))
                return hT

            # =================== layers ===================
            layer_scope = (
                tc.tile_pool(name="acts", bufs=1),
                tc.tile_pool(name="wqkv", bufs=1),
                tc.tile_pool(name="wmlp", bufs=2),
            )
            apool, wpool, mpool = [p.__enter__() for p in layer_scope]

            # persistent attention gather tiles (ones column set once)
            kg = []
            vg = []
            for g in range(KVH):
                kg.append(apool.tile([128, 4, T], BF, tag=f"kg{g}", name=f"kg{g}"))
                vgt = apool.tile([128, 4, 2, 65], BF, tag=f"vg{g}", name=f"vg{g}")
                nc.vector.memset(vgt[:, :, :, 64:65], 1.0)
                vg.append(vgt)

            sq_next = None
            for l in range(L):
                with nc.named_scope(f"layer{l}_qkv"):
                    # full-layer weights: one contiguous DMA each
                    wq_sb = wpool.tile([128, KT, KT, 128], BF, tag="wq")
                    wk_sb = wpool.tile([128, KT, 2, 128], BF, tag="wk")
                    wv_sb = wpool.tile([128, KT, 256], BF, tag="wv")
                    wo_sb = wpool.tile([64, NH, KT, 128], BF, tag="wo")
                    nc.scalar.dma_start(wk_sb[:], wk_d[l][:])
                    nc.scalar.dma_start(wv_sb[:], wv_d[l][:])
                    nc.scalar.dma_start(wq_sb[:], wq_d[l][:])
                    nc.scalar.dma_start(wo_sb[:], wo_d[l][:])

                    hT = rmsnorm(xT, sq=sq_next)
                    qT = apool.tile([128, KT, T], BF, tag="qT")
                    kT_loc = apool.tile([128, 2, T], BF, tag="kT_loc")
                    v_loc = apool.tile([128, 2, T], BF, tag="v_loc")

                    with tc.tile_pool(name="psqkv", bufs=2, space="PSUM") as pq:
                        def proj_rope(w_sl, out_sl):
                            """project one 128-feature slice and apply rope."""
                            ps = pq.tile([128, T], F32, tag="ps_qkv")
                            for kt in range(KT):
                                nc.tensor.matmul(ps[:], w_sl[:, kt], hT[:, kt],
                                                 start=(kt == 0), stop=(kt == KT - 1))
                            raw = tpool.tile([128, T], BF, tag="qraw")
                            nc.scalar.activation(raw[:], ps[:], AF.Copy)
                            rot = pq.tile([128, T], F32, tag="ps_rot")
                            nc.tensor.matmul(rot[:], ropeR_sb[:], raw[:],
                                             start=True, stop=True)
                            tcs = tpool.tile([128, T], F32, tag="tcos")
                            nc.vector.tensor_mul(out=tcs[:], in0=ps[:], in1=cos_sb[:])
                            tsn = tpool.tile([128, T], F32, tag="tsin")
                            nc.vector.tensor_mul(out=tsn[:], in0=rot[:], in1=sin_sb[:])
                            nc.vector.tensor_add(out=out_sl, in0=tcs[:], in1=tsn[:])

                        # K first, then V, so the AllGather can start early
                        for m in range(2):
                            proj_rope(wk_sb[:, :, m, :], kT_loc[:, m, :])
                        for tt in range(2):
                            psv = pq.tile([128, 256], F32, tag="ps_v")
                            for kt in range(KT):
                                nc.tensor.matmul(psv[:], hT[:, kt, tt * 128:(tt + 1) * 128],
                                                 wv_sb[:, kt],
                                                 start=(kt == 0), stop=(kt == KT - 1))
                            nc.scalar.activation(v_loc[:, tt, :], psv[:], AF.Copy)

                        # ---- AllGather K/V within group ----
                        cc_in = dpool.tile([4 * 128, T], BF, tag="cc_in")
                        cc_in_r = cc_in.rearrange("(s p) t -> p s t", p=128)
                        nc.sync.dma_start(cc_in_r[:, 0:2, :], kT_loc[:])
                        nc.sync.dma_start(cc_in_r[:, 2:4, :], v_loc[:])
                        cc_out = dpool.tile([4 * 4 * 128, T], BF, tag="cc_out")
                        nc.gpsimd.collective_compute(
                            "AllGather", mybir.AluOpType.bypass,
                            ins=[cc_in.opt()], outs=[cc_out.opt()],
                            replica_groups=GROUPS)
                        cc_r = cc_out.rearrange("(c s p) t -> p c s t", c=4, s=4)

                        # Q projection overlaps the collective
                        for m in range(KT):
                            proj_rope(wq_sb[:, :, m, :], qT[:, m, :])

                    # load gathered K/V (k rows duplicated into both halves so
                    # matmuls with q heads at base 0 or 64 have matching bases)
                    for g in range(KVH):
                        src = cc_r[64 * (g % 2):64 * (g % 2) + 64, :, g // 2, :]
                        nc.scalar.dma_start(kg[g][0:64], src)
                        nc.scalar.dma_start(kg[g][64:128], src)
                        for tt in range(2):
                            nc.scalar.dma_start(
                                vg[g][:, :, tt, 0:64],
                                cc_r[:, :, 2 + tt, g * HD:(g + 1) * HD])

                with nc.named_scope(f"layer{l}_attn"):
                    oT = apool.tile([64, NH, T], BF, tag="oT")
                    with (
                        tc.tile_pool(name="psatt", bufs=2, space="PSUM") as pa,
                        tc.tile_pool(name="psatt2", bufs=1, space="PSUM") as pb,
                        tc.tile_pool(name="pexp", bufs=3) as epool,
                    ):
                        # heads processed in even/odd pairs: their score matmuls
                        # use PE row groups 0 and 64, so interleaved issue lets
                        # the array run both concurrently (LDWEIGHTS overlaps).
                        for hp in range(NH // 2):
                            g = hp // 2
                            q0 = qT[0:64, hp, :]
                            q1 = qT[64:128, hp, :]
                            pj0s, pj1s = [], []
                            for c in range(4):
                                ps_s0 = pa.tile([128, 2, T], F32, tag="ps_s0")
                                ps_s1 = pa.tile([128, 2, T], F32, tag="ps_s1")
                                for mt in range(2):
                                    nc.tensor.matmul(
                                        ps_s0[:, mt, :],
                                        kg[g][0:64, c, mt * 128:(mt + 1) * 128],
                                        q0, start=True, stop=True)
                                    nc.tensor.matmul(
                                        ps_s1[:, mt, :],
                                        kg[g][64:128, c, mt * 128:(mt + 1) * 128],
                                        q1, start=True, stop=True)
                                e0 = epool.tile([128, 2, T], F32, tag="e0")
                                nc.scalar.activation(e0[:], ps_s0[:], AF.Exp, scale=0.125)
                                pj0 = epool.tile([128, 2, T], BF, tag="pj0")
                                nc.vector.tensor_mul(out=pj0[:], in0=e0[:],
                                                     in1=mask_sb[:, 2 * c:2 * c + 2, :])
                                pj0s.append(pj0)
                                e1 = epool.tile([128, 2, T], F32, tag="e1")
                                nc.scalar.activation(e1[:], ps_s1[:], AF.Exp, scale=0.125)
                                pj1 = epool.tile([128, 2, T], BF, tag="pj1")
                                nc.vector.tensor_mul(out=pj1[:], in0=e1[:],
                                                     in1=mask_sb[:, 2 * c:2 * c + 2, :])
                                pj1s.append(pj1)
                            # fused o + denominator (ones column -> row 64)
                            ps_o0 = pb.tile([65, T], F32, tag="ps_o0")
                            ps_o1 = pb.tile([65, T], F32, tag="ps_o1")
                            for c in range(4):
                                for tt in range(2):
                                    j = 2 * c + tt
                                    nc.tensor.matmul(ps_o0[:], vg[g][:, c, tt, :],
                                                     pj0s[c][:, tt, :],
                                                     start=(j == 0), stop=(j == 7))
                                    nc.tensor.matmul(ps_o1[:], vg[g][:, c, tt, :],
                                                     pj1s[c][:, tt, :],
                                                     start=(j == 0), stop=(j == 7))
                            for hh, ps_o in ((2 * hp, ps_o0), (2 * hp + 1, ps_o1)):
                                rec = epool.tile([65, T], BF, tag="rec")
                                with nc.allow_low_precision(reason="softmax denom"):
                                    nc.vector.reciprocal(rec[64:65, :], ps_o[64:65, :])
                                inv_ps = pb.tile([64, T], F32, tag="inv_ps")
                                nc.tensor.matmul(inv_ps[:], ones_sb[64:65, 0:64],
                                                 rec[64:65, :], start=True, stop=True)
                                inv_sb = epool.tile([64, T], F32, tag="inv_sb")
                                nc.scalar.activation(inv_sb[:], inv_ps[:], AF.Copy)
                                nc.vector.tensor_mul(out=oT[:, hh, :], in0=ps_o[0:64, :],
                                                     in1=inv_sb[:])

                    # ---- o-projection + residual (squares streamed for norm2) ----
                    sq_mlp = sqpool.tile([128, KT, T], BF, tag="sq")
                    with tc.tile_pool(name="psoproj", bufs=2, space="PSUM") as po:
                        for m in range(KT):
                            ps = po.tile([128, T], F32, tag="ps_op")
                            for hh in range(NH):
                                nc.tensor.matmul(ps[:], wo_sb[:, hh, m, :], oT[:, hh, :],
                                                 start=(hh == 0), stop=(hh == NH - 1))
                            nc.vector.tensor_add(out=xT[:, m, :], in0=xT[:, m, :], in1=ps[:])
                            nc.vector.tensor_mul(out=sq_mlp[:, m, :], in0=xT[:, m, :],
                                                 in1=xT[:, m, :])

                with nc.named_scope(f"layer{l}_mlp"):
                    h2T = rmsnorm(xT, sq=sq_mlp)
                    with (
                        tc.tile_pool(name="psmlpd", bufs=1, space="PSUM") as pmd,
                        tc.tile_pool(name="psmlp", bufs=2, space="PSUM") as pm,
                    ):
                        ps_d = [pmd.tile([128, 2, T], F32, tag=f"ps_d{i}", name=f"ps_d{i}")
                                for i in range(4)]
                        for ch in range(NCH):
                            wg_sb = mpool.tile([128, FCH, KT, 128], BF, tag="wg")
                            wu_sb = mpool.tile([128, FCH, KT, 128], BF, tag="wu")
                            wd_sb = mpool.tile([128, FCH, KT, 128], BF, tag="wd")
                            nc.sync.dma_start(wg_sb[:], wg_d[l][:, ch * FCH:(ch + 1) * FCH])
                            nc.sync.dma_start(wu_sb[:], wu_d[l][:, ch * FCH:(ch + 1) * FCH])
                            nc.sync.dma_start(wd_sb[:], wd_d[l][:, ch * FCH:(ch + 1) * FCH])
                            for fi in range(FCH):
                                f = ch * FCH + fi
                                ps_g = pm.tile([128, T], F32, tag="ps_g")
                                for kt in range(KT):
                                    nc.tensor.matmul(ps_g[:], wg_sb[:, fi, kt], h2T[:, kt],
                                                     start=(kt == 0), stop=(kt == KT - 1))
                                ps_u = pm.tile([128, T], F32, tag="ps_u")
                                for kt in range(KT):
                                    nc.tensor.matmul(ps_u[:], wu_sb[:, fi, kt], h2T[:, kt],
                                                     start=(kt == 0), stop=(kt == KT - 1))
                                silu = tpool.tile([128, T], F32, tag="silu")
                                nc.scalar.activation(silu[:], ps_g[:], AF.Silu)
                                gu = tpool.tile([128, T], BF, tag="gu")
                                nc.vector.tensor_mul(out=gu[:], in0=silu[:], in1=ps_u[:])
                                for m in range(KT):
                                    # start=True clears the WHOLE bank's has_written,
                                    # so only the first matmul touching each bank may
                                    # set it; the odd slice's first write then stores
                                    # (has_written=0) and later writes accumulate.
                                    nc.tensor.matmul(ps_d[m // 2][:, m % 2, :],
                                                     wd_sb[:, fi, m], gu[:],
                                                     start=(f == 0 and m % 2 == 0),
                                                     stop=(f == IT - 1),
                                                     skip_group_check=True)
                        sq_next = sqpool.tile([128, KT, T], BF, tag="sq")
                        for m in range(KT):
                            nc.vector.tensor_add(out=xT[:, m, :], in0=xT[:, m, :],
                                                 in1=ps_d[m // 2][:, m % 2, :])
                            nc.vector.tensor_mul(out=sq_next[:, m, :], in0=xT[:, m, :],
                                                 in1=xT[:, m, :])

            for p in reversed(layer_scope):
                p.__exit__(None, None, None)

            # =================== LM head ===================
            with nc.named_scope("lm_head"):
                hfT = rmsnorm(xT, sq=sq_next)
                cc2_in = dpool.tile([H, T], BF, tag="cc2_in")
                nc.sync.dma_start(cc2_in.rearrange("(kt p) t -> p kt t", p=128),
                                  hfT[:])
                cc2_out = dpool.tile([4 * H, T], BF, tag="cc2_out")
                nc.gpsimd.collective_compute(
                    "AllGather", mybir.AluOpType.bypass,
                    ins=[cc2_in.opt()], outs=[cc2_out.opt()],
                    replica_groups=GROUPS)
                cc2_r = cc2_out.rearrange("(c kt p) t -> p c kt t", c=4, kt=KT)

                with (
                    tc.tile_pool(name="hall", bufs=1) as hallp,
                    tc.tile_pool(name="embp", bufs=2) as embp,
                    tc.tile_pool(name="lsbp", bufs=4) as lsbp,
                    tc.tile_pool(name="pslm", bufs=4, space="PSUM") as plm,
                ):
                    hall = hallp.tile([128, 4, KT, T], BF, tag="hall")
                    nc.scalar.dma_start(hall[:], cc2_r[:])
                    for vc in range(NVC):
                        et = embp.tile([128, KT, VC], BF, tag="emb")
                        nc.sync.dma_start(et[:], emb_in[:, vc])
                        for m8 in range(8):
                            lhs = hall[:, m8 // 2, :, (m8 % 2) * 128:(m8 % 2) * 128 + 128]
                            ps = plm.tile([128, VC], F32, tag="ps_lm")
                            for kt in range(KT):
                                nc.tensor.matmul(ps[:], lhs[:, kt], et[:, kt],
                                                 start=(kt == 0), stop=(kt == KT - 1))
                            lsb = lsbp.tile([128, VC], BF, tag="lsb")
                            nc.any.tensor_copy(out=lsb[:], in_=ps[:])
                            nc.scalar.dma_start(
                                logits[m8 * 128:(m8 + 1) * 128, vc * VC:(vc + 1) * VC],
                                lsb[:])

    nc.finalize()
    return nc


# ---------------- host side ----------------

def _host_prep(inputs):
    """Build per-core input maps from full inputs."""
    ids = np.asarray(inputs["input_ids"])
    embed = np.asarray(inputs["embed"], dtype=np.float32)
    n1 = np.asarray(inputs["norm1_w"], dtype=np.float32)
    n2 = np.asarray(inputs["norm2_w"], dtype=np.float32)
    nf = np.asarray(inputs["final_norm_w"], dtype=np.float32)

    inv_freq = 1.0 / (THETA ** (np.arange(0, HD, 2, dtype=np.float64) / HD))
    R64 = np.zeros((HD, HD), np.float32)
    for i in range(32):
        R64[i, i + 32] = -1.0
        R64[i + 32, i] = 1.0
    Rblk = np.zeros((128, 128), np.float32)
    Rblk[:64, :64] = R64
    Rblk[64:, 64:] = R64
    ropeR = np.ascontiguousarray(Rblk.T).astype(BF_NP)
    ones128 = np.ones((128, 128), BF_NP)

    def prep_lhsT(w, kdim, fdim):
        """[K, F] -> [128, K/128, F/128, 128] tile layout (lhsT slices)."""
        return np.ascontiguousarray(
            w.reshape(kdim // 128, 128, fdim // 128, 128).transpose(1, 0, 2, 3)
        ).astype(BF_NP)

    # fold norm weights into following matmul weights
    common = {"ropeR": ropeR, "ones_in": ones128}
    for l in range(L):
        wq = n1[l][:, None] * np.asarray(inputs["wq"][l], np.float32)
        wk = n1[l][:, None] * np.asarray(inputs["wk"][l], np.float32)
        wv = n1[l][:, None] * np.asarray(inputs["wv"][l], np.float32)
        wo = np.asarray(inputs["wo"][l], np.float32)
        wg = n2[l][:, None] * np.asarray(inputs["w_gate"][l], np.float32)
        wu = n2[l][:, None] * np.asarray(inputs["w_up"][l], np.float32)
        wd = np.asarray(inputs["w_down"][l], np.float32)
        common[f"wq{l}"] = prep_lhsT(wq, H, H)
        common[f"wk{l}"] = prep_lhsT(wk, H, 256)
        # wv is used as matmul RHS: [128, KT, 256]
        common[f"wv{l}"] = np.ascontiguousarray(
            wv.reshape(KT, 128, 256).transpose(1, 0, 2)).astype(BF_NP)
        # wo lhsT slices are [64(d), 128(out)] per (head, m): [64, NH, KT, 128]
        common[f"wo{l}"] = np.ascontiguousarray(
            wo.reshape(NH, 64, KT, 128).transpose(1, 0, 2, 3)).astype(BF_NP)
        # MLP lhsT layouts: [128, f-tile, kt, 128]
        common[f"wg{l}"] = np.ascontiguousarray(
            wg.reshape(KT, 128, IT, 128).transpose(1, 2, 0, 3)).astype(BF_NP)
        common[f"wu{l}"] = np.ascontiguousarray(
            wu.reshape(KT, 128, IT, 128).transpose(1, 2, 0, 3)).astype(BF_NP)
        common[f"wd{l}"] = np.ascontiguousarray(
            wd.reshape(IT, 128, KT, 128).transpose(1, 0, 2, 3)).astype(BF_NP)

    in_maps = []
    for core in range(NCORE):
        b, qc = core // 4, core % 4
        pos = np.arange(T, dtype=np.float64) + qc * T
        freqs = np.outer(pos, inv_freq)
        emb = np.concatenate([freqs, freqs], axis=-1)
        cosT = np.cos(emb).T.astype(np.float32)
        sinT = np.sin(emb).T.astype(np.float32)
        mask = np.zeros((8, 128, T), np.float32)
        kvpos = np.arange(1024).reshape(8, 128)
        qpos = (np.arange(T) + qc * T)
        for j in range(8):
            mask[j] = (kvpos[j][:, None] <= qpos[None, :]).astype(np.float32)
        x0T = embed[ids[b, qc * T:(qc + 1) * T]].T          # [H, T]
        x0p = np.ascontiguousarray(
            x0T.reshape(KT, 128, T).transpose(1, 0, 2)).astype(np.float32)
        vbase = (core % 4) * VSH
        embT_shard = (nf[:, None] * embed[vbase:vbase + VSH].T)   # [H, VSH]
        embp = np.ascontiguousarray(
            embT_shard.reshape(KT, 128, NVC, VC).transpose(1, 2, 0, 3)).astype(BF_NP)
        m = dict(common)
        m.update({
            "x0": x0p,
            "cos2": np.ascontiguousarray(np.tile(cosT, (2, 1))),
            "sin2": np.ascontiguousarray(np.tile(sinT, (2, 1))),
            "mask": np.ascontiguousarray(mask.transpose(1, 0, 2)).astype(BF_NP),
            "embT": embp,
        })
        in_maps.append(m)
    return in_maps


def _get_program():
    if "prog" not in _CACHE:
        _CACHE["prog"] = build_program()
    return _CACHE["prog"]


def run(inputs, debug_layers=False, trace=False):
    nc = _get_program()
    in_maps = _host_prep(inputs)
    res = run_bass_kernel_spmd(nc, in_maps, core_ids=list(range(NCORE)), trace=trace)
    out = np.zeros((B, S, V), np.float32)
    for b in range(B):
        out[b] = np.concatenate(
            [res.results[4 * b + i]["logits"].astype(np.float32) for i in range(4)],
            axis=1)
    return out, res


def kernel(**inputs) -> np.ndarray:
    out, _ = run(inputs)
    return out


# revision 12
# speedup vs baseline: 1.0254x; 1.0254x over previous
"""Trainium2 Bass kernel for a 4-layer GQA transformer LM (nn_CustomLLM_35278861369705).

Sharding: sequence-parallel across 8 cores — 2 batch groups x 4 sequence chunks
of 256 tokens. Activations kept transposed [feature, token] on device.

v2: all matmul operands bf16 (PSUM accumulation fp32, residual stream fp32),
host pre-lays every weight into its exact SBUF tile layout so each weight
matrix is ONE contiguous large DMA (4-32KB/partition descriptors at line
rate), full-layer weights single-buffered and prefetched a layer ahead, MLP
weights streamed in double-buffered 4-ftile chunks, K/V computed before Q so
the group AllGather overlaps Q-projection, and the softmax denominator is
fused into the attention o-matmul via a ones-column appended to V.
"""
import numpy as np
import ml_dtypes

import concourse.bass as bass
import concourse.mybir as mybir
import concourse.tile as tile
from concourse import bacc
from concourse.bass_utils import run_bass_kernel_spmd

V, H, NH, KVH, I, L, S, B = 32000, 1024, 16, 4, 4096, 4, 1024, 2
HD = 64
THETA = 10000.0
EPS = 1e-5
T = 256            # tokens per core
NCORE = 8
GROUPS = [[0, 1, 2, 3], [4, 5, 6, 7]]
VSH = V // 4       # vocab shard per core (within its 4-core group)
KT = H // 128      # 8
IT = I // 128      # 32
NVC = 16           # vocab chunks per core
VC = VSH // NVC    # 500
FCH = 4            # MLP f-tiles per streamed chunk
NCH = IT // FCH    # 8

F32 = mybir.dt.float32
BF = mybir.dt.bfloat16
AF = mybir.ActivationFunctionType
BF_NP = ml_dtypes.bfloat16

_CACHE = {}


def build_program():
    nc = bacc.Bacc("TRN2", target_bir_lowering=False, debug=False,
                   num_devices=NCORE)

    # ---------------- I/O ----------------
    x0 = nc.dram_tensor("x0", [128, KT, T], F32, kind="ExternalInput").ap()
    cos2 = nc.dram_tensor("cos2", [128, T], F32, kind="ExternalInput").ap()
    sin2 = nc.dram_tensor("sin2", [128, T], F32, kind="ExternalInput").ap()
    ropeR = nc.dram_tensor("ropeR", [128, 128], BF, kind="ExternalInput").ap()
    ones_in = nc.dram_tensor("ones_in", [128, 128], BF, kind="ExternalInput").ap()
    mask_in = nc.dram_tensor("mask", [128, 8, T], BF, kind="ExternalInput").ap()
    emb_in = nc.dram_tensor("embT", [128, NVC, KT, VC], BF, kind="ExternalInput").ap()
    wq_d, wk_d, wv_d, wo_d, wg_d, wu_d, wd_d = [], [], [], [], [], [], []
    for l in range(L):
        wq_d.append(nc.dram_tensor(f"wq{l}", [128, KT, KT, 128], BF, kind="ExternalInput").ap())
        wk_d.append(nc.dram_tensor(f"wk{l}", [128, KT, 2, 128], BF, kind="ExternalInput").ap())
        wv_d.append(nc.dram_tensor(f"wv{l}", [128, KT, 256], BF, kind="ExternalInput").ap())
        wo_d.append(nc.dram_tensor(f"wo{l}", [64, NH, KT, 128], BF, kind="ExternalInput").ap())
        wg_d.append(nc.dram_tensor(f"wg{l}", [128, IT, KT, 128], BF, kind="ExternalInput").ap())
        wu_d.append(nc.dram_tensor(f"wu{l}", [128, IT, KT, 128], BF, kind="ExternalInput").ap())
        wd_d.append(nc.dram_tensor(f"wd{l}", [128, IT, KT, 128], BF, kind="ExternalInput").ap())
    logits = nc.dram_tensor("logits", [S, VSH], BF, kind="ExternalOutput").ap()

    with tile.TileContext(nc) as tc:
        with (
            tc.tile_pool(name="const", bufs=1) as cpool,
            tc.tile_pool(name="xres", bufs=1) as xpool,
            tc.tile_pool(name="hnorm", bufs=2) as hpool,
            tc.tile_pool(name="sqp", bufs=2) as sqpool,
            tc.tile_pool(name="tmps", bufs=3) as tpool,
            tc.tile_pool(name="dram", bufs=2, space="DRAM") as dpool,
        ):
            # ---- persistent constants ----
            cos_sb = cpool.tile([128, T], F32, tag="cos")
            sin_sb = cpool.tile([128, T], F32, tag="sin")
            nc.scalar.dma_start(cos_sb[:], cos2[:])
            nc.scalar.dma_start(sin_sb[:], sin2[:])
            ropeR_sb = cpool.tile([128, 128], BF, tag="ropeR")
            nc.scalar.dma_start(ropeR_sb[:], ropeR[:])
            ones_sb = cpool.tile([128, 128], BF, tag="ones")
            nc.scalar.dma_start(ones_sb[:], ones_in[:])
            mask_sb = cpool.tile([128, 8, T], BF, tag="mask")
            nc.scalar.dma_start(mask_sb[:], mask_in[:])

            # ---- residual stream ----
            xT = xpool.tile([128, KT, T], F32, tag="xT")
            nc.scalar.dma_start(xT[:], x0[:])

            def rmsnorm(src, sq=None):
                """src: [128, KT, T] f32 -> hT [128, KT, T] bf16 (norm weights are
                folded into the following matmul weights on host). If sq (the
                elementwise square of src) was already streamed out slice-wise,
                pass it in to skip the full-tile squaring here."""
                if sq is None:
                    sq = sqpool.tile([128, KT, T], BF, tag="sq")
                    nc.vector.tensor_mul(out=sq[:], in0=src[:], in1=src[:])
                with tc.tile_pool(name="psnorm", bufs=1, space="PSUM") as pp:
                    ps = pp.tile([128, T], F32, tag="ps_norm")
                    for kt in range(KT):
                        nc.tensor.matmul(ps[:], ones_sb[:], sq[:, kt],
                                         start=(kt == 0), stop=(kt == KT - 1))
                    ms = tpool.tile([128, T], F32, tag="ms")
                    nc.scalar.activation(ms[:], ps[:], AF.Copy, bias=EPS, scale=1.0 / H)
                rcp = tpool.tile([128, T], F32, tag="rcp")
                nc.vector.reciprocal(rcp[:], ms[:])
                inv = tpool.tile([128, T], F32, tag="inv")
                nc.scalar.activation(inv[:], rcp[:], AF.Sqrt)
                hT = hpool.tile([128, KT, T], BF, tag="h")
                nc.vector.tensor_mul(out=hT[:], in0=src[:],
                                     in1=inv[:, None, :].to_broadcast((128, KT, T)))
                return hT

            # =================== layers ===================
            layer_scope = (
                tc.tile_pool(name="acts", bufs=1),
                tc.tile_pool(name="wqkv", bufs=1),
                tc.tile_pool(name="wmlp", bufs=2),
            )
            apool, wpool, mpool = [p.__enter__() for p in layer_scope]

            # persistent attention gather tiles (ones column set once)
            kg = []
            vg = []
            for g in range(KVH):
                kg.append(apool.tile([128, 4, T], BF, tag=f"kg{g}", name=f"kg{g}"))
                vgt = apool.tile([128, 4, 2, 65], BF, tag=f"vg{g}", name=f"vg{g}")
                nc.vector.memset(vgt[:, :, :, 64:65], 1.0)
                vg.append(vgt)

            sq_next = None
            for l in range(L):
                with nc.named_scope(f"layer{l}_qkv"):
                    # full-layer weights: one contiguous DMA each
                    wq_sb = wpool.tile([128, KT, KT, 128], BF, tag="wq")
                    wk_sb = wpool.tile([128, KT, 2, 128], BF, tag="wk")
                    wv_sb = wpool.tile([128, KT, 256], BF, tag="wv")
                    wo_sb = wpool.tile([64, NH, KT, 128], BF, tag="wo")
                    nc.scalar.dma_start(wk_sb[:], wk_d[l][:])
                    nc.scalar.dma_start(wv_sb[:], wv_d[l][:])
                    nc.scalar.dma_start(wq_sb[:], wq_d[l][:])
                    nc.scalar.dma_start(wo_sb[:], wo_d[l][:])

                    hT = rmsnorm(xT, sq=sq_next)
                    qT = apool.tile([128, KT, T], BF, tag="qT")
                    kT_loc = apool.tile([128, 2, T], BF, tag="kT_loc")
                    v_loc = apool.tile([128, 2, T], BF, tag="v_loc")

                    with tc.tile_pool(name="psqkv", bufs=2, space="PSUM") as pq:
                        def proj_rope(w_sl, out_sl):
                            """project one 128-feature slice and apply rope."""
                            ps = pq.tile([128, T], F32, tag="ps_qkv")
                            for kt in range(KT):
                                nc.tensor.matmul(ps[:], w_sl[:, kt], hT[:, kt],
                                                 start=(kt == 0), stop=(kt == KT - 1))
                            raw = tpool.tile([128, T], BF, tag="qraw")
                            nc.scalar.activation(raw[:], ps[:], AF.Copy)
                            rot = pq.tile([128, T], F32, tag="ps_rot")
                            nc.tensor.matmul(rot[:], ropeR_sb[:], raw[:],
                                             start=True, stop=True)
                            tcs = tpool.tile([128, T], F32, tag="tcos")
                            nc.vector.tensor_mul(out=tcs[:], in0=ps[:], in1=cos_sb[:])
                            tsn = tpool.tile([128, T], F32, tag="tsin")
                            nc.vector.tensor_mul(out=tsn[:], in0=rot[:], in1=sin_sb[:])
                            nc.vector.tensor_add(out=out_sl, in0=tcs[:], in1=tsn[:])

                        # K first, then V, so the AllGather can start early
                        for m in range(2):
                            proj_rope(wk_sb[:, :, m, :], kT_loc[:, m, :])
                        for tt in range(2):
                            psv = pq.tile([128, 256], F32, tag="ps_v")
                            for kt in range(KT):
                                nc.tensor.matmul(psv[:], hT[:, kt, tt * 128:(tt + 1) * 128],
                                                 wv_sb[:, kt],
                                                 start=(kt == 0), stop=(kt == KT - 1))
                            nc.scalar.activation(v_loc[:, tt, :], psv[:], AF.Copy)

                        # ---- AllGather K/V within group ----
                        cc_in = dpool.tile([4 * 128, T], BF, tag="cc_in")
                        cc_in_r = cc_in.rearrange("(s p) t -> p s t", p=128)
                        nc.sync.dma_start(cc_in_r[:, 0:2, :], kT_loc[:])
                        nc.sync.dma_start(cc_in_r[:, 2:4, :], v_loc[:])
                        cc_out = dpool.tile([4 * 4 * 128, T], BF, tag="cc_out")
                        nc.gpsimd.collective_compute(
                            "AllGather", mybir.AluOpType.bypass,
                            ins=[cc_in.opt()], outs=[cc_out.opt()],
                            replica_groups=GROUPS)
                        cc_r = cc_out.rearrange("(c s p) t -> p c s t", c=4, s=4)

                        # Q projection overlaps the collective
                        for m in range(KT):
                            proj_rope(wq_sb[:, :, m, :], qT[:, m, :])

                    # load gathered K/V (k rows duplicated into both halves so
                    # matmuls with q heads at base 0 or 64 have matching bases)
                    for g in range(KVH):
                        src = cc_r[64 * (g % 2):64 * (g % 2) + 64, :, g // 2, :]
                        nc.scalar.dma_start(kg[g][0:64], src)
                        nc.scalar.dma_start(kg[g][64:128], src)
                        for tt in range(2):
                            nc.scalar.dma_start(
                                vg[g][:, :, tt, 0:64],
                                cc_r[:, :, 2 + tt, g * HD:(g + 1) * HD])

                with nc.named_scope(f"layer{l}_attn"):
                    oT = apool.tile([64, NH, T], BF, tag="oT")
                    with (
                        tc.tile_pool(name="psatt", bufs=3, space="PSUM") as pa,
                        tc.tile_pool(name="psatt2", bufs=1, space="PSUM") as pb,
                        tc.tile_pool(name="pexp", bufs=3) as epool,
                    ):
                        # heads processed in even/odd pairs: their score matmuls
                        # use PE row groups 0 and 64, so interleaved issue lets
                        # the array run both concurrently (LDWEIGHTS overlaps).
                        for hp in range(NH // 2):
                            g = hp // 2
                            q0 = qT[0:64, hp, :]
                            q1 = qT[64:128, hp, :]
                            pj0s, pj1s = [], []
                            for c in range(4):
                                ps_s0 = pa.tile([128, 2, T], F32, tag="ps_s0")
                                ps_s1 = pa.tile([128, 2, T], F32, tag="ps_s1")
                                for mt in range(2):
                                    nc.tensor.matmul(
                                        ps_s0[:, mt, :],
                                        kg[g][0:64, c, mt * 128:(mt + 1) * 128],
                                        q0, start=True, stop=True)
                                    nc.tensor.matmul(
                                        ps_s1[:, mt, :],
                                        kg[g][64:128, c, mt * 128:(mt + 1) * 128],
                                        q1, start=True, stop=True)
                                e0 = epool.tile([128, 2, T], F32, tag="e0")
                                nc.scalar.activation(e0[:], ps_s0[:], AF.Exp, scale=0.125)
                                pj0 = epool.tile([128, 2, T], BF, tag="pj0")
                                nc.vector.tensor_mul(out=pj0[:], in0=e0[:],
                                                     in1=mask_sb[:, 2 * c:2 * c + 2, :])
                                pj0s.append(pj0)
                                e1 = epool.tile([128, 2, T], F32, tag="e1")
                                nc.scalar.activation(e1[:], ps_s1[:], AF.Exp, scale=0.125)
                                pj1 = epool.tile([128, 2, T], BF, tag="pj1")
                                nc.vector.tensor_mul(out=pj1[:], in0=e1[:],
                                                     in1=mask_sb[:, 2 * c:2 * c + 2, :])
                                pj1s.append(pj1)
                            # fused o + denominator (ones column -> row 64)
                            ps_o0 = pb.tile([65, T], F32, tag="ps_o0")
                            ps_o1 = pb.tile([65, T], F32, tag="ps_o1")
                            for c in range(4):
                                for tt in range(2):
                                    j = 2 * c + tt
                                    nc.tensor.matmul(ps_o0[:], vg[g][:, c, tt, :],
                                                     pj0s[c][:, tt, :],
                                                     start=(j == 0), stop=(j == 7))
                                    nc.tensor.matmul(ps_o1[:], vg[g][:, c, tt, :],
                                                     pj1s[c][:, tt, :],
                                                     start=(j == 0), stop=(j == 7))
                            for hh, ps_o in ((2 * hp, ps_o0), (2 * hp + 1, ps_o1)):
                                rec = epool.tile([65, T], BF, tag="rec")
                                with nc.allow_low_precision(reason="softmax denom"):
                                    nc.vector.reciprocal(rec[64:65, :], ps_o[64:65, :])
                                # partition-broadcast 1/denominator via SBUF->SBUF DMA
                                inv_sb = epool.tile([64, T], BF, tag="inv_sb")
                                nc.scalar.dma_start(inv_sb[:],
                                                    rec[64:65, None, :].to_broadcast((1, 64, T)))
                                nc.vector.tensor_mul(out=oT[:, hh, :], in0=ps_o[0:64, :],
                                                     in1=inv_sb[:])

                    # ---- o-projection + residual (squares streamed for norm2) ----
                    sq_mlp = sqpool.tile([128, KT, T], BF, tag="sq")
                    with tc.tile_pool(name="psoproj", bufs=2, space="PSUM") as po:
                        for m in range(KT):
                            ps = po.tile([128, T], F32, tag="ps_op")
                            for hh in range(NH):
                                nc.tensor.matmul(ps[:], wo_sb[:, hh, m, :], oT[:, hh, :],
                                                 start=(hh == 0), stop=(hh == NH - 1))
                            nc.vector.tensor_add(out=xT[:, m, :], in0=xT[:, m, :], in1=ps[:])
                            nc.vector.tensor_mul(out=sq_mlp[:, m, :], in0=xT[:, m, :],
                                                 in1=xT[:, m, :])

                with nc.named_scope(f"layer{l}_mlp"):
                    h2T = rmsnorm(xT, sq=sq_mlp)
                    with (
                        tc.tile_pool(name="psmlpd", bufs=1, space="PSUM") as pmd,
                        tc.tile_pool(name="psmlp", bufs=2, space="PSUM") as pm,
                    ):
                        ps_d = [pmd.tile([128, 2, T], F32, tag=f"ps_d{i}", name=f"ps_d{i}")
                                for i in range(4)]
                        for ch in range(NCH):
                            wg_sb = mpool.tile([128, FCH, KT, 128], BF, tag="wg")
                            wu_sb = mpool.tile([128, FCH, KT, 128], BF, tag="wu")
                            wd_sb = mpool.tile([128, FCH, KT, 128], BF, tag="wd")
                            nc.sync.dma_start(wg_sb[:], wg_d[l][:, ch * FCH:(ch + 1) * FCH])
                            nc.sync.dma_start(wu_sb[:], wu_d[l][:, ch * FCH:(ch + 1) * FCH])
                            nc.sync.dma_start(wd_sb[:], wd_d[l][:, ch * FCH:(ch + 1) * FCH])
                            for fi in range(FCH):
                                f = ch * FCH + fi
                                ps_g = pm.tile([128, T], F32, tag="ps_g")
                                for kt in range(KT):
                                    nc.tensor.matmul(ps_g[:], wg_sb[:, fi, kt], h2T[:, kt],
                                                     start=(kt == 0), stop=(kt == KT - 1))
                                ps_u = pm.tile([128, T], F32, tag="ps_u")
                                for kt in range(KT):
                                    nc.tensor.matmul(ps_u[:], wu_sb[:, fi, kt], h2T[:, kt],
                                                     start=(kt == 0), stop=(kt == KT - 1))
                                silu = tpool.tile([128, T], F32, tag="silu")
                                nc.scalar.activation(silu[:], ps_g[:], AF.Silu)
                                gu = tpool.tile([128, T], BF, tag="gu")
                                nc.vector.tensor_mul(out=gu[:], in0=silu[:], in1=ps_u[:])
                                for m in range(KT):
                                    # start=True clears the WHOLE bank's has_written,
                                    # so only the first matmul touching each bank may
                                    # set it; the odd slice's first write then stores
                                    # (has_written=0) and later writes accumulate.
                                    nc.tensor.matmul(ps_d[m // 2][:, m % 2, :],
                                                     wd_sb[:, fi, m], gu[:],
                                                     start=(f == 0 and m % 2 == 0),
                                                     stop=(f == IT - 1),
                                                     skip_group_check=True)
                        sq_next = sqpool.tile([128, KT, T], BF, tag="sq")
                        for m in range(KT):
                            nc.vector.tensor_add(out=xT[:, m, :], in0=xT[:, m, :],
                                                 in1=ps_d[m // 2][:, m % 2, :])
                            nc.vector.tensor_mul(out=sq_next[:, m, :], in0=xT[:, m, :],
                                                 in1=xT[:, m, :])

            for p in reversed(layer_scope):
                p.__exit__(None, None, None)

            # =================== LM head ===================
            with nc.named_scope("lm_head"):
                hfT = rmsnorm(xT, sq=sq_next)
                cc2_in = dpool.tile([H, T], BF, tag="cc2_in")
                nc.sync.dma_start(cc2_in.rearrange("(kt p) t -> p kt t", p=128),
                                  hfT[:])
                cc2_out = dpool.tile([4 * H, T], BF, tag="cc2_out")
                nc.gpsimd.collective_compute(
                    "AllGather", mybir.AluOpType.bypass,
                    ins=[cc2_in.opt()], outs=[cc2_out.opt()],
                    replica_groups=GROUPS)
                cc2_r = cc2_out.rearrange("(c kt p) t -> p c kt t", c=4, kt=KT)

                with (
                    tc.tile_pool(name="hall", bufs=1) as hallp,
                    tc.tile_pool(name="embp", bufs=2) as embp,
                    tc.tile_pool(name="lsbp", bufs=4) as lsbp,
                    tc.tile_pool(name="pslm", bufs=4, space="PSUM") as plm,
                ):
                    hall = hallp.tile([128, 4, KT, T], BF, tag="hall")
                    nc.scalar.dma_start(hall[:], cc2_r[:])
                    for vc in range(NVC):
                        et = embp.tile([128, KT, VC], BF, tag="emb")
                        nc.sync.dma_start(et[:], emb_in[:, vc])
                        for m8 in range(8):
                            lhs = hall[:, m8 // 2, :, (m8 % 2) * 128:(m8 % 2) * 128 + 128]
                            ps = plm.tile([128, VC], F32, tag="ps_lm")
                            for kt in range(KT):
                                nc.tensor.matmul(ps[:], lhs[:, kt], et[:, kt],
                                                 start=(kt == 0), stop=(kt == KT - 1))
                            lsb = lsbp.tile([128, VC], BF, tag="lsb")
                            nc.any.tensor_copy(out=lsb[:], in_=ps[:])
                            nc.scalar.dma_start(
                                logits[m8 * 128:(m8 + 1) * 128, vc * VC:(vc + 1) * VC],
                                lsb[:])

    nc.finalize()
    return nc


# ---------------- host side ----------------

def _host_prep(inputs):
    """Build per-core input maps from full inputs."""
    ids = np.asarray(inputs["input_ids"])
    embed = np.asarray(inputs["embed"], dtype=np.float32)
    n1 = np.asarray(inputs["norm1_w"], dtype=np.float32)
    n2 = np.asarray(inputs["norm2_w"], dtype=np.float32)
    nf = np.asarray(inputs["final_norm_w"], dtype=np.float32)

    inv_freq = 1.0 / (THETA ** (np.arange(0, HD, 2, dtype=np.float64) / HD))
    R64 = np.zeros((HD, HD), np.float32)
    for i in range(32):
        R64[i, i + 32] = -1.0
        R64[i + 32, i] = 1.0
    Rblk = np.zeros((128, 128), np.float32)
    Rblk[:64, :64] = R64
    Rblk[64:, 64:] = R64
    ropeR = np.ascontiguousarray(Rblk.T).astype(BF_NP)
    ones128 = np.ones((128, 128), BF_NP)

    def prep_lhsT(w, kdim, fdim):
        """[K, F] -> [128, K/128, F/128, 128] tile layout (lhsT slices)."""
        return np.ascontiguousarray(
            w.reshape(kdim // 128, 128, fdim // 128, 128).transpose(1, 0, 2, 3)
        ).astype(BF_NP)

    # fold norm weights into following matmul weights
    common = {"ropeR": ropeR, "ones_in": ones128}
    for l in range(L):
        wq = n1[l][:, None] * np.asarray(inputs["wq"][l], np.float32)
        wk = n1[l][:, None] * np.asarray(inputs["wk"][l], np.float32)
        wv = n1[l][:, None] * np.asarray(inputs["wv"][l], np.float32)
        wo = np.asarray(inputs["wo"][l], np.float32)
        wg = n2[l][:, None] * np.asarray(inputs["w_gate"][l], np.float32)
        wu = n2[l][:, None] * np.asarray(inputs["w_up"][l], np.float32)
        wd = np.asarray(inputs["w_down"][l], np.float32)
        common[f"wq{l}"] = prep_lhsT(wq, H, H)
        common[f"wk{l}"] = prep_lhsT(wk, H, 256)
        # wv is used as matmul RHS: [128, KT, 256]
        common[f"wv{l}"] = np.ascontiguousarray(
            wv.reshape(KT, 128, 256).transpose(1, 0, 2)).astype(BF_NP)
        # wo lhsT slices are [64(d), 128(out)] per (head, m): [64, NH, KT, 128]
        common[f"wo{l}"] = np.ascontiguousarray(
            wo.reshape(NH, 64, KT, 128).transpose(1, 0, 2, 3)).astype(BF_NP)
        # MLP lhsT layouts: [128, f-tile, kt, 128]
        common[f"wg{l}"] = np.ascontiguousarray(
            wg.reshape(KT, 128, IT, 128).transpose(1, 2, 0, 3)).astype(BF_NP)
        common[f"wu{l}"] = np.ascontiguousarray(
            wu.reshape(KT, 128, IT, 128).transpose(1, 2, 0, 3)).astype(BF_NP)
        common[f"wd{l}"] = np.ascontiguousarray(
            wd.reshape(IT, 128, KT, 128).transpose(1, 0, 2, 3)).astype(BF_NP)

    in_maps = []
    for core in range(NCORE):
        b, qc = core // 4, core % 4
        pos = np.arange(T, dtype=np.float64) + qc * T
        freqs = np.outer(pos, inv_freq)
        emb = np.concatenate([freqs, freqs], axis=-1)
        cosT = np.cos(emb).T.astype(np.float32)
        sinT = np.sin(emb).T.astype(np.float32)
        mask = np.zeros((8, 128, T), np.float32)
        kvpos = np.arange(1024).reshape(8, 128)
        qpos = (np.arange(T) + qc * T)
        for j in range(8):
            mask[j] = (kvpos[j][:, None] <= qpos[None, :]).astype(np.float32)
        x0T = embed[ids[b, qc * T:(qc + 1) * T]].T          # [H, T]
        x0p = np.ascontiguousarray(
            x0T.reshape(KT, 128, T).transpose(1, 0, 2)).astype(np.float32)
        vbase = (core % 4) * VSH
        embT_shard = (nf[:, None] * embed[vbase:vbase + VSH].T)   # [H, VSH]
        embp = np.ascontiguousarray(
            embT_shard.reshape(KT, 128, NVC, VC).transpose(1, 2, 0, 3)).astype(BF_NP)
        m = dict(common)
        m.update({
            "x0": x0p,
            "cos2": np.ascontiguousarray(np.tile(cosT, (2, 1))),
            "sin2": np.ascontiguousarray(np.tile(sinT, (2, 1))),
            "mask": np.ascontiguousarray(mask.transpose(1, 0, 2)).astype(BF_NP),
            "embT": embp,
        })
        in_maps.append(m)
    return in_maps


def _get_program():
    if "prog" not in _CACHE:
        _CACHE["prog"] = build_program()
    return _CACHE["prog"]


def run(inputs, debug_layers=False, trace=False):
    nc = _get_program()
    in_maps = _host_prep(inputs)
    res = run_bass_kernel_spmd(nc, in_maps, core_ids=list(range(NCORE)), trace=trace)
    out = np.zeros((B, S, V), np.float32)
    for b in range(B):
        out[b] = np.concatenate(
            [res.results[4 * b + i]["logits"].astype(np.float32) for i in range(4)],
            axis=1)
    return out, res


def kernel(**inputs) -> np.ndarray:
    out, _ = run(inputs)
    return out
